# revision 25
# baseline (speedup 1.0000x reference)
"""MultiHeadContrastive loss on 8 TRN2 NeuronCores (Bass/Tile SPMD).

Strategy: data-parallel over the anchor (row) dimension, with the global
row order permuted host-side so background rows (label==0) come first.
Each core owns N/8 = 1024 rows. The two heads are pipelined: the fg-head
MLP/projection runs first and its (fp8-quantized, transposed) embeddings
are AllGathered immediately (~13us in), so the collective entry cost
overlaps the cls-head MLP; the fg exp loop then overlaps the cls
AllGather.

The NxN work runs in a row-major layout (own-i on partitions, j on free):
PSUM sim tiles [128, 2048] are exp'd by the scalar engine, with row sums
produced by ACT accum_out on 1 of 4 tiles and by vector-engine reduces on
the rest (accum_out costs a READ_ACCUMULATOR slot on the ACT queue, and
the DVE is otherwise idle), so no accumulation matmuls are needed and the
loop is purely ACT-bound. The fg-only numerator uses S_fg = S_all - S_bg,
where S_bg is a masked DVE reduce of the first 512 (bg-sorted) columns of
the already-computed exp tile — no extra sims or exp work. Supcon
positive-pair sums use linearity via the class-mean embedding (one-hot
matmul + AllReduce).
"""
import numpy as np
import ml_dtypes

import concourse.bacc as bacc
import concourse.mybir as mybir
import concourse.tile as tile
import concourse.bass_utils as bass_utils
from concourse.tile_rust import add_dep_helper

NCORES = 8
N, C, H, DF, DC = 8192, 1024, 256, 64, 128
HC = 2 * H            # both heads' hidden, concatenated
SH = N // NCORES      # 1024 rows per core
NIC = SH // 128       # 8 i-chunks of 128 rows
NCLS = 21
EPS = 1e-8
TAU = 0.2
NBGT = 512            # bg-masked tile width (must cover n_bg)
NJB = N // 2048       # 4 j-blocks of 2048 per (ic, head)

BF16 = mybir.dt.bfloat16
F32 = mybir.dt.float32
FP8 = mybir.dt.float8e4
AF = mybir.ActivationFunctionType
ALU = mybir.AluOpType

_cached = {}


def _build():
    nc = bacc.Bacc("TRN2", target_bir_lowering=False, debug=False,
                   num_devices=NCORES)

    def inp(name, shape, dt):
        return nc.dram_tensor(name, shape, dt, kind="ExternalInput")

    xT = inp("xT", [C, SH], BF16)            # own rows (bg-sorted), transposed
    w1 = inp("w1", [C, HC], BF16)            # [fg_w1 | cls_w1]
    b1 = inp("b1", [128, HC // 128], F32)    # partition-major
    w2f = inp("w2f", [H, DF], BF16)
    w2c = inp("w2c", [H, DC], BF16)
    b2f8 = inp("b2f8", [128, NIC * DF], F32)   # fg b2 bcast per i-chunk
    b2c8 = inp("b2c8", [128, NIC * DC], F32)   # cls b2 bcast per i-chunk
    fgown = inp("fgown", [128, NIC], F32)    # own fg mask
    iou = inp("iou", [128, NIC], F32)        # own ious
    ohb = inp("ohb", [128, NIC * NCLS], BF16)  # own-label one-hot per i-chunk
    ident = inp("ident", [128, 128], BF16)
    identF = inp("identF", [128, 128], F32)
    bgmask = inp("bgmask", [128, NBGT], BF16)  # 1 for bg cols, 0 for fg cols

    psums = nc.dram_tensor("psums", [1, 8], F32, kind="ExternalOutput")
    dbg = nc.dram_tensor("dbg", [128, 64], F32, kind="ExternalOutput")

    # collective buffers
    zpackF = nc.dram_tensor("zpackF", [DF, SH], FP8)
    zgathF = nc.dram_tensor("zgathF", [NCORES * DF, SH], FP8,
                            addr_space="Shared")
    zpackC = nc.dram_tensor("zpackC", [DC, SH], FP8)
    zgathC = nc.dram_tensor("zgathC", [NCORES * DC, SH], FP8,
                            addr_space="Shared")
    cbL = nc.dram_tensor("cbL", [NCLS, DC + 1], F32)
    cbR = nc.dram_tensor("cbR", [NCLS, DC + 1], F32, addr_space="Shared")

    rg = [list(range(NCORES))]

    with tile.TileContext(nc) as tc:
        with (
            tc.tile_pool(name="persist", bufs=1) as P,
            tc.tile_pool(name="work", bufs=2) as W,
            tc.tile_pool(name="exps", bufs=2) as EX,
        ):
            # ---- load persistent inputs into SBUF ----
            xT_sb = P.tile([128, (C // 128) * SH], BF16, tag="xT")
            xT_r = xT.ap().rearrange("(c p) r -> p c r", p=128)
            w1_sb = P.tile([128, (C // 128) * HC], BF16, tag="w1")
            w1_r = w1.ap().rearrange("(c p) h -> p c h", p=128)
            for c in range(C // 128):
                nc.sync.dma_start(w1_sb[:, c * HC:(c + 1) * HC],
                                  w1_r[:, c:c + 1, :])
                nc.sync.dma_start(xT_sb[:, c * SH:(c + 1) * SH],
                                  xT_r[:, c:c + 1, :])
            b1_sb = P.tile([128, HC // 128], F32, tag="b1")
            nc.sync.dma_start(b1_sb[:, :], b1.ap())
            w2f_sb = P.tile([128, (H // 128) * DF], BF16, tag="w2f")
            nc.sync.dma_start(w2f_sb[:, :], w2f.ap().rearrange(
                "(m p) d -> p m d", p=128))
            w2c_sb = P.tile([128, (H // 128) * DC], BF16, tag="w2c")
            nc.sync.dma_start(w2c_sb[:, :], w2c.ap().rearrange(
                "(m p) d -> p m d", p=128))
            b2f8_sb = P.tile([128, NIC * DF], F32, tag="b2f8")
            nc.sync.dma_start(b2f8_sb[:, :], b2f8.ap())
            b2c8_sb = P.tile([128, NIC * DC], F32, tag="b2c8")
            nc.sync.dma_start(b2c8_sb[:, :], b2c8.ap())
            fgown_sb = P.tile([128, NIC], F32, tag="fgown")
            nc.sync.dma_start(fgown_sb[:, :], fgown.ap())
            iou_sb = P.tile([128, NIC], F32, tag="iou")
            nc.sync.dma_start(iou_sb[:, :], iou.ap())
            ohb_sb = P.tile([128, NIC * NCLS], BF16, tag="ohb")
            nc.sync.dma_start(ohb_sb[:, :], ohb.ap())
            ident_sb = P.tile([128, 128], BF16, tag="ident")
            nc.sync.dma_start(ident_sb[:, :], ident.ap())
            identF_sb = P.tile([128, 128], F32, tag="identF")
            nc.sync.dma_start(identF_sb[:, :], identF.ap())
            bgmask_sb = P.tile([128, NBGT], BF16, tag="bgmask")
            nc.sync.dma_start(bgmask_sb[:, :], bgmask.ap())

            onesR_sb = P.tile([1, 128], F32, tag="onesR")    # outer-product lhsT
            nc.vector.memset(onesR_sb[:, :], 1.0)
            onesP_sb = P.tile([128, 1], F32, tag="onesP")    # final reduce lhsT
            nc.vector.memset(onesP_sb[:, :], 1.0)
            eps2_sb = P.tile([128, 1], F32, tag="eps2")
            nc.vector.memset(eps2_sb[:, :], 2.0 * EPS)
            eps1_sb = P.tile([128, 1], F32, tag="eps1")
            nc.vector.memset(eps1_sb[:, :], EPS)
            # ACT table warmup: force the ln/exp table load off the
            # critical path (overlaps the MLP matmuls)
            warm_sb = P.tile([1, 2], F32, tag="warm")
            nc.scalar.activation(warm_sb[:, 0:1], onesR_sb[0:1, 0:1], AF.Ln)
            nc.scalar.activation(warm_sb[:, 1:2], onesR_sb[0:1, 0:1], AF.Exp)

            # persistent SBUF results
            hT_sb = P.tile([128, (HC // 128) * SH], BF16, tag="hT")
            zfq_sb = P.tile([128, NIC * DF], BF16, tag="zfq")
            zcq_sb = P.tile([128, NIC * (DC + 1)], BF16, tag="zcq")
            znfT_q = P.tile([64, SH], BF16, tag="znfTq")
            zncT_q = P.tile([128, SH], BF16, tag="zncTq")
            znfT8 = P.tile([64, SH], FP8, tag="znfT8")
            zncT8 = P.tile([128, SH], FP8, tag="zncT8")
            ssqf_sb = P.tile([128, NIC], F32, tag="ssqf")
            ssqc_sb = P.tile([128, NIC], F32, tag="ssqc")
            spos_sb = P.tile([128, NIC], F32, tag="spos")
            npos_sb = P.tile([128, NIC], F32, tag="npos")
            zfT_all = P.tile([64, N], FP8, tag="zfT_all")
            zcT_all = P.tile([128, N], FP8, tag="zcT_all")
            cb_sb = P.tile([NCLS, DC + 1], F32, tag="cb_sb")
            cbl_sb = P.tile([NCLS, DC + 1], F32, tag="cbl_sb")
            zbcT_sb = P.tile([128, NCLS], BF16, tag="zbcT_sb")
            hist_sb = P.tile([1, NCLS], F32, tag="hist_sb")
            fgtot_sb = P.tile([1, 1], F32, tag="fgtot")
            histB_sb = P.tile([128, NCLS], F32, tag="histB")
            ftB_sb = P.tile([128, 1], F32, tag="ftB")
            # exp row-sum partials (accum_out targets)
            pf_sb = P.tile([128, NIC * NJB], F32, tag="pf")
            pc_sb = P.tile([128, NIC * NJB], F32, tag="pc")
            pbg_sb = P.tile([128, NIC], F32, tag="pbg")

            # ---- phase 1a: fg hidden = relu(w1f.T @ xT + b1f) ----
            PH1ctx = tc.tile_pool(name="ph1a", bufs=1, space="PSUM")
            PH1 = PH1ctx.__enter__()
            for m in range(2):                  # fg H-chunks
                pq = [PH1.tile([128, 256], F32, tag=f"hps{q}",
                               name=f"hps{q}", bufs=(2 if q < 3 else 1))
                      for q in range(4)]
                for c in range(C // 128):       # 8 K-chunks
                    for q in range(4):          # 4x N=256 per LDW
                        nc.tensor.matmul(
                            pq[q][:, :],
                            lhsT=w1_sb[:, c * HC + m * 128:c * HC + (m + 1) * 128],
                            rhs=xT_sb[:, c * SH + q * 256:c * SH + q * 256 + 256],
                            start=(c == 0), stop=(c == C // 128 - 1))
                for q in range(4):
                    nc.vector.tensor_scalar(
                        hT_sb[:, m * SH + q * 256:m * SH + q * 256 + 256],
                        pq[q][:, :], b1_sb[:, m:m + 1], 0.0,
                        ALU.add, ALU.max)
            PH1ctx.__exit__(None, None, None)

            # ---- phase 2a: fg z, normalize, quantize, transpose, AG ----
            PZfctx = tc.tile_pool(name="pzf", bufs=1, space="PSUM")
            PZf = PZfctx.__enter__()
            zf_ps = PZf.tile([128, NIC * DF], F32, tag="zf")
            for ic in range(NIC):
                for hm in range(2):
                    nc.tensor.matmul(
                        zf_ps[:, ic * DF:(ic + 1) * DF],
                        lhsT=hT_sb[:, hm * SH + ic * 128:hm * SH + ic * 128 + 128],
                        rhs=w2f_sb[:, hm * DF:(hm + 1) * DF],
                        start=(hm == 0), stop=(hm == 1))
            ztf = P.tile([128, NIC * DF], F32, tag="ztf")
            nc.vector.tensor_add(ztf[:, :], zf_ps[:, :], b2f8_sb[:, :])
            PZfctx.__exit__(None, None, None)
            sqf = W.tile([128, NIC * DF], F32, tag="sqf", name="sqf")
            nc.vector.tensor_mul(sqf[:, :], ztf[:, :], ztf[:, :])
            n2f = P.tile([128, NIC], F32, tag="n2f")
            nc.vector.tensor_reduce(
                n2f[:, :], sqf[:, :].rearrange("p (i c) -> p i c", i=NIC),
                mybir.AxisListType.X, ALU.add)
            lnvf = P.tile([128, NIC], F32, tag="lnvf")
            nc.scalar.activation(lnvf[:, :], n2f[:, :], AF.Ln)
            ninvf = P.tile([128, NIC], F32, tag="ninvf")
            nc.scalar.activation(ninvf[:, :], lnvf[:, :], AF.Exp, scale=-0.5)
            zf8 = P.tile([128, NIC * DF], FP8, tag="zf8")
            for ic in range(NIC):
                nc.vector.tensor_scalar_mul(
                    zf8[:, ic * DF:(ic + 1) * DF],
                    ztf[:, ic * DF:(ic + 1) * DF], ninvf[:, ic:ic + 1])
            nc.vector.tensor_copy(zfq_sb[:, :], zf8[:, :])
            PTfctx = tc.tile_pool(name="ptf", bufs=1, space="PSUM")
            PTf = PTfctx.__enter__()
            for ic in range(NIC):
                zfT_ps = PTf.tile([64, 128], BF16, tag="ztrf",
                                  name="zfT_ps", bufs=2)
                nc.tensor.transpose(zfT_ps[:, :],
                                    zfq_sb[:, ic * DF:(ic + 1) * DF],
                                    ident_sb[:, :])
                nc.vector.tensor_copy(znfT_q[:, ic * 128:(ic + 1) * 128],
                                      zfT_ps[:, :])
            PTfctx.__exit__(None, None, None)
            nc.vector.tensor_copy(znfT8[:, :], znfT_q[:, :])
            nc.sync.dma_start(zpackF.ap(), znfT8[:, :])
            agF = nc.gpsimd.collective_compute(
                "AllGather", ALU.bypass, replica_groups=rg,
                ins=[zpackF.ap().opt()], outs=[zgathF.ap().opt()])
            for r in range(NCORES):
                # alternate DMA queues so unloads run pairwise-parallel
                eng = nc.sync if r % 2 == 0 else nc.scalar
                eng.dma_start(
                    zfT_all[:, r * SH:(r + 1) * SH],
                    zgathF.ap()[r * DF:(r + 1) * DF, :])
            # ssq of the quantized fg z (matches the PE diagonal sims)
            sq2f = W.tile([128, NIC * DF], F32, tag="sqf", name="sq2f")
            nc.vector.tensor_mul(sq2f[:, :], zfq_sb[:, :], zfq_sb[:, :])
            nc.vector.tensor_reduce(
                ssqf_sb[:, :], sq2f[:, :].rearrange("p (i c) -> p i c", i=NIC),
                mybir.AxisListType.X, ALU.add)

            # ---- phase 1b: cls hidden ----
            PH1bctx = tc.tile_pool(name="ph1b", bufs=1, space="PSUM")
            PH1b = PH1bctx.__enter__()
            for m in range(2, 4):               # cls H-chunks
                pq = [PH1b.tile([128, 256], F32, tag=f"hps{q}",
                               name=f"hps{q}", bufs=(2 if q < 3 else 1))
                      for q in range(4)]
                for c in range(C // 128):
                    for q in range(4):
                        nc.tensor.matmul(
                            pq[q][:, :],
                            lhsT=w1_sb[:, c * HC + m * 128:c * HC + (m + 1) * 128],
                            rhs=xT_sb[:, c * SH + q * 256:c * SH + q * 256 + 256],
                            start=(c == 0), stop=(c == C // 128 - 1))
                for q in range(4):
                    nc.vector.tensor_scalar(
                        hT_sb[:, m * SH + q * 256:m * SH + q * 256 + 256],
                        pq[q][:, :], b1_sb[:, m:m + 1], 0.0,
                        ALU.add, ALU.max)
            PH1bctx.__exit__(None, None, None)

            # ---- phase 2b: cls z, normalize, quantize, transpose, CB ----
            PZcctx = tc.tile_pool(name="pzc", bufs=1, space="PSUM")
            PZc = PZcctx.__enter__()
            zc_ps = PZc.tile([128, NIC * DC], F32, tag="zc")
            for ic in range(NIC):
                for hm in range(2):
                    nc.tensor.matmul(
                        zc_ps[:, ic * DC:(ic + 1) * DC],
                        lhsT=hT_sb[:, (2 + hm) * SH + ic * 128:(2 + hm) * SH + ic * 128 + 128],
                        rhs=w2c_sb[:, hm * DC:(hm + 1) * DC],
                        start=(hm == 0), stop=(hm == 1))
            ztc = P.tile([128, NIC * DC], F32, tag="ztc")
            nc.vector.tensor_add(ztc[:, :], zc_ps[:, :], b2c8_sb[:, :])
            PZcctx.__exit__(None, None, None)
            sqc = W.tile([128, NIC * DC], F32, tag="sqc", name="sqc")
            nc.vector.tensor_mul(sqc[:, :], ztc[:, :], ztc[:, :])
            n2c = P.tile([128, NIC], F32, tag="n2c")
            nc.vector.tensor_reduce(
                n2c[:, :], sqc[:, :].rearrange("p (i c) -> p i c", i=NIC),
                mybir.AxisListType.X, ALU.add)
            lnvc = P.tile([128, NIC], F32, tag="lnvc")
            nc.scalar.activation(lnvc[:, :], n2c[:, :], AF.Ln)
            ninvc = P.tile([128, NIC], F32, tag="ninvc")
            nc.scalar.activation(ninvc[:, :], lnvc[:, :], AF.Exp, scale=-0.5)
            zc8 = P.tile([128, NIC * DC], FP8, tag="zc8")
            for ic in range(NIC):
                nc.vector.tensor_scalar_mul(
                    zc8[:, ic * DC:(ic + 1) * DC],
                    ztc[:, ic * DC:(ic + 1) * DC], ninvc[:, ic:ic + 1])
            zcq_v = zcq_sb[:, :].rearrange("p (i c) -> p i c", i=NIC)
            nc.vector.tensor_copy(
                zcq_v[:, :, 0:DC],
                zc8[:, :].rearrange("p (i c) -> p i c", i=NIC))
            nc.vector.memset(zcq_v[:, :, DC:DC + 1], 1.0)
            PTcctx = tc.tile_pool(name="ptc", bufs=1, space="PSUM")
            PTc = PTcctx.__enter__()
            PCctx = tc.tile_pool(name="pcb", bufs=1, space="PSUM")
            PC = PCctx.__enter__()
            cb_ps = PC.tile([NCLS, DC + 1], F32, tag="cb")
            for ic in range(NIC):
                zoff = ic * (DC + 1)
                nc.tensor.matmul(
                    cb_ps[:, :],
                    lhsT=ohb_sb[:, ic * NCLS:(ic + 1) * NCLS],
                    rhs=zcq_sb[:, zoff:zoff + DC + 1],
                    start=(ic == 0), stop=(ic == NIC - 1))
                zcT_ps = PTc.tile([128, 128], BF16, tag="ztrc",
                                  name="zcT_ps", bufs=2)
                nc.tensor.transpose(zcT_ps[:, :],
                                    zcq_sb[:, zoff:zoff + DC],
                                    ident_sb[:, :])
                nc.vector.tensor_copy(zncT_q[:, ic * 128:(ic + 1) * 128],
                                      zcT_ps[:, :])
            nc.vector.tensor_copy(zncT8[:, :], zncT_q[:, :])
            nc.sync.dma_start(zpackC.ap(), zncT8[:, :])
            agC = nc.gpsimd.collective_compute(
                "AllGather", ALU.bypass, replica_groups=rg,
                ins=[zpackC.ap().opt()], outs=[zgathC.ap().opt()])
            add_dep_helper(agC.ins, agF.ins, reason="AGf before AGc")
            nc.vector.tensor_copy(cbl_sb[:, :], cb_ps[:, :])
            nc.sync.dma_start(cbL.ap(), cbl_sb[:, :])
            ar_inst = nc.gpsimd.collective_compute(
                "AllReduce", ALU.add, replica_groups=rg,
                ins=[cbL.ap().opt()], outs=[cbR.ap().opt()])
            add_dep_helper(ar_inst.ins, agC.ins, reason="AGc before AR")
            for r in range(NCORES):
                nc.sync.dma_start(
                    zcT_all[:, r * SH:(r + 1) * SH],
                    zgathC.ap()[r * DC:(r + 1) * DC, :])
            PCctx.__exit__(None, None, None)
            PTcctx.__exit__(None, None, None)
            sq2c = W.tile([128, NIC * DC], F32, tag="sqc", name="sq2c")
            zcq2_v = zcq_sb[:, :].rearrange("p (i c) -> p i c", i=NIC)
            nc.vector.tensor_mul(
                sq2c[:, :].rearrange("p (i c) -> p i c", i=NIC),
                zcq2_v[:, :, 0:DC], zcq2_v[:, :, 0:DC])
            nc.vector.tensor_reduce(
                ssqc_sb[:, :], sq2c[:, :].rearrange("p (i c) -> p i c", i=NIC),
                mybir.AxisListType.X, ALU.add)

            # accum-independent small tensors (fill AG wait)
            edf_sb = P.tile([128, NIC], F32, tag="edf_sb")
            nc.scalar.activation(edf_sb[:, :], ssqf_sb[:, :], AF.Exp,
                                 scale=1.0 / TAU)
            edc_sb = P.tile([128, NIC], F32, tag="edc_sb")
            nc.scalar.activation(edc_sb[:, :], ssqc_sb[:, :], AF.Exp,
                                 scale=1.0 / TAU)
            t0f = P.tile([128, NIC], F32, tag="t0f")
            nc.vector.tensor_mul(t0f[:, :], edf_sb[:, :], fgown_sb[:, :])
            iouw_pre = P.tile([128, NIC], F32, tag="iouw_pre")
            thr0 = W.tile([128, NIC], F32, tag="thr0", name="thr0")
            nc.vector.tensor_scalar(thr0[:, :], iou_sb[:, :], -0.5, 1e9,
                                    ALU.add, ALU.mult)
            nc.vector.tensor_scalar_max(thr0[:, :], thr0[:, :], 0.0)
            nc.vector.tensor_scalar_min(thr0[:, :], thr0[:, :], 1.0)
            nc.vector.tensor_mul(iouw_pre[:, :], iou_sb[:, :], thr0[:, :])

            # ---- phase 5: exp loops (ACT-bound) ----
            with tc.tile_pool(name="psim", bufs=2, space="PSUM") as PJ:
                # fg head, then cls head (cls AG overlaps the fg loop)
                for hd in range(2):
                    zT_own = (znfT8 if hd == 0 else zncT8)
                    zT_all = (zfT_all if hd == 0 else zcT_all)
                    psum_t = (pf_sb if hd == 0 else pc_sb)
                    for ic in range(NIC):
                        for jb in range(NJB):
                            ps = PJ.tile([128, 2048], F32, tag="sim",
                                         name="sim")
                            for q in range(4):
                                nc.tensor.matmul(
                                    ps[:, q * 512:(q + 1) * 512],
                                    lhsT=zT_own[:, ic * 128:(ic + 1) * 128],
                                    rhs=zT_all[:, jb * 2048 + q * 512:
                                               jb * 2048 + (q + 1) * 512],
                                    start=True, stop=True)
                            ex = EX.tile([128, 2048], BF16, tag="ex",
                                         name="ex", bufs=4)
                            slot = psum_t[:, ic * NJB + jb:ic * NJB + jb + 1]
                            if jb == NJB - 1 or (hd == 1 and ic == NIC - 1):
                                # accum_out costs a READ_ACCUMULATOR slot on
                                # the ACT queue; keep it on 1 of 4 tiles and
                                # row-sum the rest on the idle vector engine.
                                # The final group is all-accum so no DVE
                                # reduce backlog trails the last exp call.
                                nc.scalar.activation(
                                    ex[:, :], ps[:, :], AF.Exp,
                                    scale=1.0 / TAU, accum_out=slot)
                            else:
                                nc.scalar.activation(
                                    ex[:, :], ps[:, :], AF.Exp,
                                    scale=1.0 / TAU)
                                nc.vector.tensor_reduce(
                                    slot,
                                    ex[:, :].rearrange("p (a b) -> p a b",
                                                       a=1),
                                    mybir.AxisListType.X, ALU.add)
                            if hd == 0 and jb == 0:
                                # S_bg straight from the exp values of the
                                # first (bg-sorted) columns: mask and reduce
                                # on the vector engine; no extra ACT work
                                em = W.tile([128, NBGT], BF16, tag="em",
                                            name="em", bufs=4)
                                nc.vector.tensor_mul(em[:, :],
                                                     ex[:, 0:NBGT],
                                                     bgmask_sb[:, :])
                                nc.vector.tensor_reduce(
                                    pbg_sb[:, ic:ic + 1],
                                    em[:, :].rearrange("p (a b) -> p a b",
                                                       a=1),
                                    mybir.AxisListType.X, ALU.add)

            # ---- phase 4: zbar / hist prep + spos/npos (needs AR) ----
            with tc.tile_pool(name="p4", bufs=1, space="PSUM") as P4:
                nc.sync.dma_start(cb_sb[:, :], cbR.ap())
                zbcT_ps = P4.tile([128, NCLS], F32, tag="ps4", name="zbcT_ps",
                                  bufs=2)
                nc.tensor.transpose(zbcT_ps[:, :], cb_sb[:, 0:DC],
                                    identF_sb[0:NCLS, 0:NCLS])
                nc.vector.tensor_copy(zbcT_sb[:, :], zbcT_ps[:, :])
                hist_ps = P4.tile([1, NCLS], F32, tag="ps4", name="hist_ps",
                                  bufs=2)
                nc.tensor.transpose(hist_ps[:, :], cb_sb[:, DC:DC + 1],
                                    identF_sb[0:NCLS, 0:NCLS])
                nc.vector.tensor_copy(hist_sb[:, :], hist_ps[:, :])
                nc.vector.tensor_reduce(fgtot_sb[:, :], hist_sb[:, :],
                                        mybir.AxisListType.X, ALU.add)
                hb_ps = P4.tile([128, NCLS + 1], F32, tag="ps4", name="hb_ps",
                                bufs=2)
                nc.tensor.matmul(hb_ps[:, 0:NCLS], lhsT=onesR_sb[:, :],
                                 rhs=hist_sb[:, :], start=True, stop=True)
                nc.tensor.matmul(hb_ps[:, NCLS:NCLS + 1], lhsT=onesR_sb[:, :],
                                 rhs=fgtot_sb[:, :], start=True, stop=True)
                nc.vector.tensor_copy(histB_sb[:, :], hb_ps[:, 0:NCLS])
                nc.vector.tensor_copy(ftB_sb[:, :], hb_ps[:, NCLS:NCLS + 1])

                gall_ps = P4.tile([128, NIC * 32], F32, tag="gall")
                for ic in range(NIC):
                    nc.tensor.matmul(gall_ps[:, ic * 32:ic * 32 + NCLS],
                                     lhsT=zncT_q[:, ic * 128:(ic + 1) * 128],
                                     rhs=zbcT_sb[:, :], start=True, stop=True)
                g_v = gall_ps[:, :].rearrange("p (i c) -> p i c", i=NIC)
                oh_v = ohb_sb[:, :].rearrange("p (i c) -> p i c", i=NIC)
                gm = W.tile([128, NIC * NCLS], F32, tag="gm")
                gm_v = gm[:, :].rearrange("p (i c) -> p i c", i=NIC)
                nc.vector.tensor_mul(gm_v, g_v[:, :, 0:NCLS], oh_v)
                nc.vector.tensor_reduce(spos_sb[:, :], gm_v,
                                        mybir.AxisListType.X, ALU.add)
                hb8 = W.tile([128, NIC * NCLS], F32, tag="hb8")
                for r in range(NIC):
                    nc.vector.tensor_copy(hb8[:, r * NCLS:(r + 1) * NCLS],
                                          histB_sb[:, :])
                nm = W.tile([128, NIC * NCLS], F32, tag="nm")
                nm_v = nm[:, :].rearrange("p (i c) -> p i c", i=NIC)
                nc.vector.tensor_mul(
                    nm_v, hb8[:, :].rearrange("p (i c) -> p i c", i=NIC), oh_v)
                nc.vector.tensor_reduce(npos_sb[:, :], nm_v,
                                        mybir.AxisListType.X, ALU.add)

                # accum-independent final-phase terms
                nposf = W.tile([128, NIC], F32, tag="nposf", name="nposf")
                nc.vector.tensor_scalar(nposf[:, :], fgown_sb[:, :], -1.0,
                                        ftB_sb[:, 0:1], ALU.mult, ALU.add)
                vf = W.tile([128, NIC], F32, tag="vf", name="vf")
                nc.vector.tensor_scalar_min(vf[:, :], nposf[:, :], 1.0)
                validf = W.tile([128, NIC], F32, tag="validf", name="validf")
                nc.vector.tensor_mul(validf[:, :], vf[:, :], fgown_sb[:, :])
                FIN = P.tile([128, 32], F32, tag="FIN")
                nc.vector.tensor_mul(FIN[:, 8:16], iouw_pre[:, :],
                                     validf[:, :])
                vc = W.tile([128, NIC], F32, tag="vc", name="vc")
                nc.vector.tensor_scalar_min(vc[:, :], npos_sb[:, :], 1.0)
                validc = W.tile([128, NIC], F32, tag="validc", name="validc")
                nc.vector.tensor_mul(validc[:, :], vc[:, :], fgown_sb[:, :])
                nc.vector.tensor_mul(FIN[:, 24:32], iouw_pre[:, :],
                                     validc[:, :])
                t2m = P.tile([128, NIC], F32, tag="t2m")
                nc.vector.tensor_sub(t2m[:, :], spos_sb[:, :], ssqc_sb[:, :])
                nc.vector.tensor_scalar(t2m[:, :], t2m[:, :], -1.0 / TAU, 1e9,
                                        ALU.mult, ALU.add)
                npm1 = P.tile([128, NIC], F32, tag="npm1s")
                nc.vector.tensor_scalar_add(npm1[:, :], npos_sb[:, :], -1.0)
                hh = W.tile([128, NIC], F32, tag="hh", name="hh")
                nc.vector.tensor_scalar_add(hh[:, :], npos_sb[:, :], EPS)
                rcp_sb = P.tile([128, NIC], F32, tag="rcp_sb")
                nc.vector.reciprocal(rcp_sb[:, :], hh[:, :])

            # ---- phase 6: final assembly ----
            with tc.tile_pool(name="pfin", bufs=2, space="PSUM") as PF:
                Sf = P.tile([128, NIC], F32, tag="Sf")
                nc.vector.tensor_reduce(
                    Sf[:, :],
                    pf_sb[:, :].rearrange("p (i q) -> p i q", q=NJB),
                    mybir.AxisListType.X, ALU.add)
                Sc = P.tile([128, NIC], F32, tag="Sc")
                nc.vector.tensor_reduce(
                    Sc[:, :],
                    pc_sb[:, :].rearrange("p (i q) -> p i q", q=NJB),
                    mybir.AxisListType.X, ALU.add)
                def T(tag):
                    return W.tile([128, NIC], F32, tag=tag, name=tag)

                denom = T("denom")
                nc.vector.tensor_sub(denom[:, :], Sf[:, :], edf_sb[:, :])
                numer = T("numer")
                nc.vector.tensor_sub(numer[:, :], Sf[:, :], pbg_sb[:, :])
                nc.vector.tensor_sub(numer[:, :], numer[:, :], t0f[:, :])
                denc = T("denc")
                nc.vector.tensor_sub(denc[:, :], Sc[:, :], edc_sb[:, :])
                lnd = T("lnd")
                nc.scalar.activation(lnd[:, :], denom[:, :], AF.Ln,
                                     bias=eps2_sb[:, 0:1])
                lnn = T("lnn")
                nc.scalar.activation(lnn[:, :], numer[:, :], AF.Ln,
                                     bias=eps1_sb[:, 0:1])
                lndc = T("lndc")
                nc.scalar.activation(lndc[:, :], denc[:, :], AF.Ln)
                lossf = T("lossf")
                nc.vector.tensor_sub(lossf[:, :], lnd[:, :], lnn[:, :])
                nc.vector.tensor_mul(FIN[:, 0:8], FIN[:, 8:16], lossf[:, :])
                t3 = T("t3")
                nc.vector.tensor_mul(t3[:, :], npm1[:, :], lndc[:, :])
                g = T("g")
                nc.vector.tensor_add(g[:, :], t2m[:, :], t3[:, :])
                lzi = T("lzi")
                nc.vector.tensor_mul(lzi[:, :], g[:, :], rcp_sb[:, :])
                nc.vector.tensor_mul(FIN[:, 16:24], FIN[:, 24:32], lzi[:, :])

                fin_ps = PF.tile([1, 32], F32, tag="fin")
                nc.tensor.matmul(fin_ps[:, :], lhsT=onesP_sb[:, :],
                                 rhs=FIN[:, :], start=True, stop=True)
                res4 = P.tile([1, 8], F32, tag="res4")
                nc.vector.tensor_reduce(
                    res4[:, 0:4],
                    fin_ps[:, :].rearrange("p (q c) -> p q c", q=4),
                    mybir.AxisListType.X, ALU.add)
                nc.vector.tensor_copy(res4[:, 4:5], fgtot_sb[:, :])
                nc.vector.memset(res4[:, 5:8], 0.0)
                nc.sync.dma_start(psums.ap(), res4[:, :])

                for k, t in enumerate([Sf, Sc, pbg_sb, ssqf_sb, ssqc_sb,
                                       spos_sb, npos_sb, lzi]):
                    nc.sync.dma_start(dbg.ap()[:, k * 8:(k + 1) * 8],
                                      t[:, :])

    nc.compile()
    return nc


def _prep_inputs(roi_feats, labels, ious, fg_w1, fg_b1, fg_w2, fg_b2,
                 cls_w1, cls_b1, cls_w2, cls_b2):
    bf = ml_dtypes.bfloat16
    labels = np.asarray(labels).astype(np.int64)
    ious = np.asarray(ious, np.float32)
    roi = np.asarray(roi_feats, np.float32)

    # global bg-first sort (loss sums are permutation-invariant)
    perm = np.argsort(labels != 0, kind="stable")
    labels = labels[perm]
    ious = ious[perm]
    roi = roi[perm]
    n_bg = int((labels == 0).sum())
    assert n_bg <= NBGT, f"bg count {n_bg} exceeds masked tile {NBGT}"

    w1cat = np.concatenate([np.asarray(fg_w1), np.asarray(cls_w1)],
                           axis=1).astype(bf)                      # [C, 512]
    b1cat = np.concatenate([np.asarray(fg_b1), np.asarray(cls_b1)])
    b1pm = np.ascontiguousarray(
        b1cat.reshape(HC // 128, 128).T).astype(np.float32)        # [128, 4]
    b2f8 = np.tile(np.tile(np.asarray(fg_b2, np.float32), (128, 1)),
                   (1, NIC))                                       # [128, 8*64]
    b2c8 = np.tile(np.tile(np.asarray(cls_b2, np.float32), (128, 1)),
                   (1, NIC))                                       # [128, 8*128]

    fg_glob = (labels > 0).astype(np.float32)                      # [N]
    ident = np.eye(128, dtype=np.float32)

    bgmask = np.zeros((128, NBGT), np.float32)
    bgmask[:, :n_bg] = 1.0

    # one-hot of labels, label 0 excluded
    oh_glob = np.zeros((N, NCLS), np.float32)
    oh_glob[np.arange(N), labels % NCLS] = (labels > 0)

    in_maps = []
    for k in range(NCORES):
        sl = slice(k * SH, (k + 1) * SH)
        oh_own = oh_glob[sl]                                       # [1024, 21]
        ohb = np.concatenate(
            [oh_own[ic * 128:(ic + 1) * 128] for ic in range(NIC)],
            axis=1).astype(bf)                                     # [128, 8*21]
        in_maps.append({
            "xT": np.ascontiguousarray(roi[sl].T).astype(bf),
            "w1": w1cat,
            "b1": b1pm,
            "w2f": np.asarray(fg_w2).astype(bf),
            "w2c": np.asarray(cls_w2).astype(bf),
            "b2f8": b2f8,
            "b2c8": b2c8,
            "fgown": np.ascontiguousarray(
                fg_glob[sl].reshape(NIC, 128).T).astype(np.float32),
            "iou": np.ascontiguousarray(
                ious[sl].reshape(NIC, 128).T).astype(np.float32),
            "ohb": ohb,
            "ident": ident.astype(bf),
            "identF": ident,
            "bgmask": bgmask.astype(bf),
        })
    return in_maps


def _get_nc():
    if "nc" not in _cached:
        _cached["nc"] = _build()
    return _cached["nc"]


def run(inputs, trace=False, tmpdir=None):
    nc = _get_nc()
    in_maps = _prep_inputs(**inputs)
    res = bass_utils.run_bass_kernel_spmd(
        nc, in_maps, core_ids=list(range(NCORES)), trace=trace, tmpdir=tmpdir)
    swl_f = sw_f = swl_c = sw_c = 0.0
    for r in res.results:
        p = r["psums"][0].astype(np.float64)
        swl_f += p[0]; sw_f += p[1]; swl_c += p[2]; sw_c += p[3]
    loss_fg = swl_f / (sw_f + EPS)
    loss_c = swl_c / (sw_c + EPS)
    out = np.array([loss_fg, loss_c], np.float32)
    return out, res


def kernel(**inputs) -> np.ndarray:
    out, _ = run(inputs)
    return out
